# revision 4
# baseline (speedup 1.0000x reference)
"""HDClassifier Trainium2 kernel, v4 — j-interleaved layout, 6 k-passes.

Same architecture as v3 (see its docstring) but sharded GB=4 x GD=2:
each core handles 8 batches x 5000 d-columns. With only 8 batches per
group, the number of DISTINCT (channel, level) pairs actually used is
<= 1536 = 6*256 (vs 1608 worst case), so the host compacts the pair
space per group and phase A needs only 6 fp8-DoubleRow k-passes instead
of 7: PE drops from 59.7us to 51.2us. Falls back to KP=7 if some group
uses more pairs.

Per-core cost-model budget: PE 51.2us, DMA ~55us (7.7 table + 1.5 oh +
1.6 halo + 10.0 gram out, mostly overlapped), DVE ~49us, ACT ~42us.
"""

import sys

sys.path.insert(0, "/opt/trn_rl_repo")

import numpy as np

import concourse.bass as bass
import concourse.mybir as mybir
from concourse import bacc
from concourse.bass_utils import run_bass_kernel_spmd
from concourse.tile import TileContext

# Problem constants
NUM_LEVELS = 201
B, T, C, D = 32, 128, 8, 10000
N_CORES = 8
GB, GD = 4, 2
B_LOC = B // GB            # 8 batches per core
W = D // GD                # 5000 output cols per core
W3 = W + 3                 # incl. left halo
NJ = 40                    # d interleave: dl = NJ*p + j
NP = 128
K_TOT = C * NUM_LEVELS     # 1608
COLS = B_LOC * T           # 1024
NTP = T - 3                # 125 valid t' per batch
SHIP_P = 125               # dl = NJ*p + j < 5000 -> p <= 124

FP8 = mybir.dt.float8e4
BF16 = mybir.dt.bfloat16
F32 = mybir.dt.float32
NP_FP8 = np.dtype(mybir.dt.np(FP8))
NP_BF16 = np.dtype(mybir.dt.np(BF16))

_CACHE = {}

# j processed 0-3 (halos+head) then 36-39 (so the +2-chained late grams
# have their high-j u's early, and PE has tab-ready work while the
# deferred table stream lands); tail after the last drain (b_35) is just
# u_34, u_35 and grams 32..35.
JORDER = [0, 1, 2, 3, 37, 38, 39, 36] + list(range(4, 36))
SHIP_GROUPS = [(36, 40), (0, 4), (4, 8), (8, 12), (12, 16), (16, 20),
               (20, 24), (24, 28), (28, 30), (30, 32), (32, 34), (34, 35),
               (35, 36)]
GRP_OF = {}
for _gi, (_j0, _j1) in enumerate(SHIP_GROUPS):
    for _j in range(_j0, _j1):
        GRP_OF[_j] = _gi


def _build_program(KP):
    nc = bacc.Bacc("TRN2", target_bir_lowering=False, debug=False, num_devices=N_CORES)

    tab_p = nc.declare_dram_parameter("tab", [128, NJ, KP, 2, NP], FP8, isOutput=False)
    oh_p = nc.declare_dram_parameter("oh", [128, KP, 2, COLS], FP8, isOutput=False)
    gr_p = nc.declare_dram_parameter("gr", [NJ, SHIP_P, B_LOC * NTP], BF16, isOutput=True)

    with TileContext(nc) as tc:
        with (
            tc.tile_pool(name="const", bufs=1) as cpool,
            tc.tile_pool(name="b", bufs=6) as bpool,
            tc.tile_pool(name="u", bufs=8) as upool,
            tc.tile_pool(name="gram", bufs=3) as gpool,
            tc.tile_pool(name="ps", bufs=4, space="PSUM") as pspool,
        ):
            tab = cpool.tile([128, NJ, KP, 2, NP], FP8, tag="tab")
            oh = cpool.tile([128, KP, 2, COLS], FP8, tag="oh")

            # Loads: one-hot kp-blocks on the ACT HWDGE queue, early table
            # j-blocks on SP (the two queues' descriptor gens interleave on
            # the shared HWDGE unit, bus follows ready order); tab[36:40]
            # gated behind the oh kp2 block via a reservation copy (RAW on
            # the oh region, WAW on the tab region). The bulk table blocks
            # are emitted inside the pipeline after the halos.
            nc.sync.dma_start(out=tab[:, 0:2], in_=tab_p[:, 0:2])
            nc.scalar.dma_start(out=oh[:, 0], in_=oh_p[:, 0])
            nc.sync.dma_start(out=tab[:, 2:4], in_=tab_p[:, 2:4])
            for kp in range(1, KP):
                nc.scalar.dma_start(out=oh[:, kp], in_=oh_p[:, kp])
            nc.gpsimd.tensor_copy(out=tab[0:1, 37, 0, 0, 0:2], in_=oh[0:1, 1, 0, 0:2])
            nc.sync.dma_start(out=tab[:, 37:40], in_=tab_p[:, 37:40])
            nc.gpsimd.tensor_copy(out=tab[0:1, 36, 0, 0, 0:2], in_=oh[0:1, 2, 0, 0:2])
            nc.sync.dma_start(out=tab[:, 36:37], in_=tab_p[:, 36:37])

            bt = {}      # j -> b tile (0..NJ-1), NJ..NJ+2 are halos
            ut = {}      # j -> (u tile, n_partitions)
            gtiles = {}
            gdone = set()

            def mm_j(j, ps):
                for g in range(2):
                    for kp in range(KP):
                        nc.tensor.matmul(
                            ps[:, g * 512:(g + 1) * 512],
                            tab[:, j, kp],
                            oh[:, kp, :, g * 512:(g + 1) * 512],
                            start=(kp == 0),
                            stop=(kp == KP - 1),
                            perf_mode=mybir.MatmulPerfMode.DoubleRow,
                        )

            def drain_j(j, ps):
                b = bpool.tile([128, COLS], BF16, tag="b", name=f"b{j}")
                nc.scalar.copy(out=b[:, 0:COLS], in_=ps[:])
                bt[j] = b
                if j < 3:
                    # halo: b_{NJ+j}[p] = b_j[p+1] (partition-shift DMA;
                    # on the ACT queue right after its producing drain, so
                    # its sem-wait barely parks the SEQ)
                    hl = cpool.tile([127, COLS], BF16, tag=f"halo{j}", name=f"halo{j}")
                    nc.scalar.dma_start(out=hl[:], in_=b[1:128, 0:COLS])
                    bt[NJ + j] = hl
                if j == 2:
                    # bulk table loads, queue-ordered behind the halos
                    for j0 in range(4, 36, 6):
                        j1 = min(j0 + 6, 36)
                        nc.scalar.dma_start(out=tab[:, j0:j1], in_=tab_p[:, j0:j1])

            def try_u(j):
                if j in ut or j not in bt or j + 1 not in bt:
                    return
                if j >= NJ and NJ - 1 not in ut:
                    # halo-u's are only needed by the late grams; emitting
                    # them early head-of-line-blocks the DVE queue on the
                    # halo DMAs
                    return
                np_ = 128 if j + 1 < NJ else 127
                # col COLS-1 of u is never read by any gram (in-block t'+2
                # stays <= 1022), so writing COLS-1 cols avoids reading any
                # pad col of b -- no pads, no memsets anywhere.
                u = upool.tile([128, COLS], BF16, tag="u", name=f"u{j}")
                nc.vector.tensor_mul(
                    out=u[0:np_, 0:COLS - 1],
                    in0=bt[j][0:np_, 0:COLS - 1],
                    in1=bt[j + 1][0:np_, 1:COLS],
                )
                ut[j] = (u, np_)

            def gram_ship(j):
                gi = GRP_OF[j]
                j0, j1 = SHIP_GROUPS[gi]
                if gi not in gtiles:
                    gtiles[gi] = gpool.tile(
                        [SHIP_P, (j1 - j0) * B_LOC * NTP], BF16,
                        tag=f"gram{j1 - j0}", name=f"g{gi}"
                    )
                g = gtiles[gi]
                jj = j - j0
                u0, _ = ut[j]
                u2, _ = ut[j + 2]
                in0 = u0[0:SHIP_P, 0:COLS].rearrange("p (b t) -> p b t", b=B_LOC)[:, :, 0:NTP]
                in1 = u2[0:SHIP_P, 0:COLS].rearrange("p (b t) -> p b t", b=B_LOC)[:, :, 2:NTP + 2]
                out = g[:, jj * B_LOC * NTP:(jj + 1) * B_LOC * NTP].rearrange(
                    "p (b t) -> p b t", b=B_LOC
                )
                # Mid-run grams run on the otherwise-idle Pool engine
                # (~2.1us each vs 0.59 on DVE, but it trims DVE's tail
                # backlog, which sets the finish time).
                eng = nc.gpsimd if j in POOL_G else nc.vector
                eng.tensor_mul(out=out, in0=in0, in1=in1)
                if len([x for x in range(j0, j1) if x in gdone]) == j1 - j0 - 1:
                    # Tail ships go on the (idle-by-then) ACT queue: 632ns
                    # descriptor gen vs ~1.1us on Pool.
                    eng = nc.scalar if gi >= len(SHIP_GROUPS) - 3 else nc.sync
                    eng.dma_start(
                        out=gr_p[j0:j1].rearrange("j p c -> p j c"),
                        in_=g[:].rearrange("p (j c) -> p j c", j=j1 - j0),
                    )

            TAIL_GRAMS = (32, 33)
            POOL_G = frozenset({7, 9, 11, 13, 15, 17, 19, 21, 32})

            def emit_grams(only=None, waterfall=False):
                for j in (only if only is not None else range(NJ)):
                    if j not in gdone and j in ut and j + 2 in ut:
                        gram_ship(j)
                        gdone.add(j)
                    elif waterfall and j not in gdone:
                        break

            def advance():
                # Interleave: after each new u, immediately emit the
                # tail-critical grams it unlocks IN SHIP-GROUP ORDER
                # (waterfall: don't jump ahead of a not-yet-ready gram, so
                # each ship group completes as early as possible).
                emit_grams()
                for j in range(NJ + 2):
                    before = j in ut
                    try_u(j)
                    if not before and j in ut:
                        emit_grams(TAIL_GRAMS, waterfall=True)
                emit_grams()

            # Head: kp-major over j0-1 then j2-3 so PE starts on the first
            # one-hot kp-block and the first drains land early.
            HEAD = JORDER[:4]
            head_ps = {j: pspool.tile([128, COLS], F32, tag="ps", name=f"psh{j}") for j in HEAD}
            for jpair in (HEAD[0:2], HEAD[2:4]):
                for kp in range(KP):
                    for j in jpair:
                        for g in range(2):
                            nc.tensor.matmul(
                                head_ps[j][:, g * 512:(g + 1) * 512],
                                tab[:, j, kp],
                                oh[:, kp, :, g * 512:(g + 1) * 512],
                                start=(kp == 0),
                                stop=(kp == KP - 1),
                                perf_mode=mybir.MatmulPerfMode.DoubleRow,
                            )
                for j in jpair:
                    drain_j(j, head_ps[j])
                    advance()

            for j in JORDER[4:]:
                ps = pspool.tile([128, COLS], F32, tag="ps", name=f"ps{j}")
                mm_j(j, ps)
                drain_j(j, ps)
                advance()
            advance()
            assert len(gdone) == NJ, f"grams stuck: {sorted(set(range(NJ)) - gdone)}"

    nc.finalize()
    return nc


def _host_prep(x, level_hv, channel_hv):
    # Bit-exact replication of the jax fp32 quantization
    x = np.asarray(x, dtype=np.float32)
    t1 = x + np.float32(100.0)
    t2 = t1 / np.float32(200.0)
    t3 = t2 * np.float32(200.0)
    idx = np.clip(np.rint(t3), 0, NUM_LEVELS - 1).astype(np.int32)  # [B,T,C]

    fp8_one = np.float32(1.0).astype(NP_FP8)
    fp8_mone = np.float32(-1.0).astype(NP_FP8)

    # folded +-1 table [K_TOT, D] fp8, pair id k = c*201 + level
    prod = (channel_hv[:, None, :] * level_hv[None, :, :]).reshape(K_TOT, D)
    tabf = np.where(prod > 0, fp8_one, fp8_mone)

    # Per-group pair compaction: with 8 batches/group the used-pair count
    # is <= 1536 (checked dynamically), enabling 6 k-passes.
    cc = np.arange(C)[None, None, :]
    kk_all = cc * NUM_LEVELS + idx                      # [B, T, C]
    groups = []
    for gb in range(GB):
        kk = kk_all[gb * B_LOC:(gb + 1) * B_LOC]        # [B_LOC, T, C]
        used = np.unique(kk)
        groups.append((kk, used))
    n_max = max(len(u) for _, u in groups)
    KP = 6 if n_max <= 6 * 256 else 7
    K_PAD = KP * 256

    ohs, tabs = [], []
    for gb in range(GB):
        kk, used = groups[gb]
        slot = np.full(K_TOT, 0, np.int32)
        slot[used] = np.arange(len(used))
        # one-hot on compacted slots: [K_PAD, COLS]
        oh = np.zeros((K_PAD, COLS), dtype=NP_FP8)
        bb, tt, ccg = np.meshgrid(np.arange(B_LOC), np.arange(T), np.arange(C), indexing="ij")
        oh[slot[kk].ravel(), (bb * T + tt).ravel()] = fp8_one
        ohs.append(np.ascontiguousarray(
            oh.reshape(KP, 2, 128, COLS).transpose(2, 0, 1, 3)))
        # compacted table rows for this group, per d-window
        row = np.zeros((K_PAD, D), dtype=NP_FP8)
        row[:len(used)] = tabf[used]
        per_gd = []
        for gd in range(GD):
            dls = NJ * np.arange(NP)[None, :] + np.arange(NJ)[:, None]  # [NJ, NP]
            cols = (gd * W - 3 + dls) % D
            tcore = np.zeros((K_PAD, NJ, NP), dtype=NP_FP8)
            valid = dls < W3
            tcore[:, valid] = row[:, cols[valid]]
            per_gd.append(np.ascontiguousarray(
                tcore.reshape(KP, 2, 128, NJ, NP).transpose(2, 3, 0, 1, 4)))
        tabs.append(per_gd)
    return KP, tabs, ohs


def kernel(x, level_hv, channel_hv, centroid):
    KP, tabs, ohs = _host_prep(x, level_hv, channel_hv)
    if ("nc", KP) not in _CACHE:
        _CACHE[("nc", KP)] = _build_program(KP)
    nc = _CACHE[("nc", KP)]
    _CACHE["nc"] = nc

    in_maps = []
    for core in range(N_CORES):
        gb, gd = divmod(core, GD)
        in_maps.append({"tab": tabs[gb][gd], "oh": ohs[gb]})

    res = run_bass_kernel_spmd(nc, in_maps, list(range(N_CORES)))
    _CACHE["last_results"] = res

    sample = np.zeros((B, D), dtype=np.float32)
    dl = NJ * np.arange(SHIP_P)[None, :] + np.arange(NJ)[:, None]  # [NJ, P]
    ok = dl < W
    for core in range(N_CORES):
        gb, gd = divmod(core, GD)
        gr = res.results[core]["gr"]  # [NJ, SHIP_P, B_LOC*NTP] bf16
        g32 = (np.ascontiguousarray(gr).view(np.uint16).astype(np.uint32) << 16).view(np.float32)
        s = g32.reshape(NJ, SHIP_P, B_LOC, NTP).sum(axis=-1)  # [NJ, P, B_LOC]
        for b2 in range(B_LOC):
            row = np.zeros(W, dtype=np.float32)
            row[dl[ok]] = s[:, :, b2][ok]
            sample[gb * B_LOC + b2, gd * W:(gd + 1) * W] = row
    sign = np.where(sample > 0, np.float32(1.0), np.float32(-1.0))
    return (sign @ np.asarray(centroid, dtype=np.float32).T).astype(np.float32)
